# revision 70
# baseline (speedup 1.0000x reference)
# kernel.py — Trainium2 Bass kernel for nn_DenseGridNet (bilinear grid sample + MLP)
#
# Strategy (data-parallel over 8 NeuronCores, sorted sharding):
#  * host: computes integer cell ids, sorts points by cell (routing only),
#    builds a 256B-padded patch table PT[cell] = [v00,v10,v01,v11,pad] and
#    int16 window-local gather indices (dma_gather needs int16; windows of
#    32768 rows are fixed at compile time from uniform quantiles).
#  * device: per 8192-point chunk: bulk dma_gather of 256B patch rows;
#    DVE computes bilinear weights/coefs from u,v and builds fp16
#    q = coef*patch (+ idf + ones cols); one xbar DMA transpose flips
#    point-major -> feature-major; TensorE runs the 3 MLP layers as
#    block-diagonal matmuls (2 pts/col); DVE+ACT do relu / sigmoid+bias
#    PSUM->SBUF passes; output lands as yT [6, B/2] and is un-permuted on host.
import os
import numpy as np

RX = 1024
RY = 1024
F = 4
HID = 64
N_CORES = 8
P = 128          # partitions
SLOT = 32        # fp16 columns per point in the pre-transpose buffer
QROWS = 18       # used rows per point: 16 q + idf + ones
WIN = 16384      # dma_gather window rows per chunk (int16-addressable)
CHPTS = 16384    # points per gather/transpose/MLP chunk
PATCH_F16 = True     # patch table dtype (False -> fp32, 64B payload)
POOL_OPS = True      # offload part of q-build + idf copy to the Pool engine
PTROW = 128 if PATCH_F16 else 64   # patch row STRIDE in elements (256B)
GROW = 16        # gathered payload per point, elements
QC = 512         # columns per sigmoid group in the packed yT layout


def _dma_gather_strided(g, out_ap, in_ap, idxs_ap, num_idxs, elem_size, elem_step):
    """dma_gather with elem_size smaller than the 256B-multiple the bass
    wrapper asserts. The hardware encodes the source stride separately
    (stride_bytes_256, 256B units) from the transfer size (elem_size), so a
    64B payload gathered from 256B-strided rows is representable; only the
    wrapper's transpose-mode assert blocks it. Mirrors BassGpSimd.dma_gather
    (non-transpose, DRAM source, gen_mode=0)."""
    import concourse.mybir as mybir
    from concourse import ap_utils

    assert idxs_ap.dtype == mybir.dt.int16
    assert in_ap.dtype == out_ap.dtype
    assert in_ap.ap[0][0] == elem_step
    assert in_ap.ap[-1][1] == out_ap.ap[-1][1] == elem_size
    assert out_ap.ap[0][1] * out_ap.ap[1][1] * out_ap.ap[2][1] // elem_size >= num_idxs
    stride_bytes = elem_step * mybir.dt.size(in_ap.dtype)
    assert stride_bytes % 256 == 0
    stride_bytes_256 = stride_bytes // 256
    _in_ap = g.lower_ap_dma(in_ap, for_custom_bir_dma=True)
    _idxs_ap = g.lower_ap(idxs_ap)
    _out_ap = g.lower_ap(out_ap)
    return g.add_instruction(
        mybir.InstDMAGatherAnt(
            name=g.bass.get_next_instruction_name(),
            ins=[*_in_ap, _idxs_ap, g.lower_val_access(g.to_reg(num_idxs))],
            outs=[_out_ap],
            transpose=False,
            num_idxs=num_idxs,
            elem_size=elem_size,
            stride_bytes_256=stride_bytes_256,
            gen_mode=0,
            single_packet=False,
            queue_num=0,
            sbuf_tokens_per_rank=0,
            sbuf_free_dim_per_rank=0,
            sbuf_free_dim_pad_per_rank=0,
            sbuf_byte_offset=0,
        )
    )


def _build_bass(B, n_cores, core0_chunk0):
    """Bass program for one core processing B sorted points.

    core0_chunk0: list of per-core first-global-chunk indices is not needed;
    windows differ per core, but SPMD shares one program — so window bases
    must be identical across cores. Instead the table input is pre-shifted
    per core on host: in_ ap uses window base relative to a per-core table
    slice. We pass windows as compile-time offsets into the per-core table
    input "ptw" which the host fills with the right 32768-row windows.
    """
    import concourse.bass as bass
    import concourse.tile as tile
    from concourse import bacc, library_config
    import concourse.mybir as mybir

    dt = mybir.dt
    T = B // P
    n_chunks = B // CHPTS
    ch_cols = CHPTS // P            # 64 point cols per chunk
    n_blk = ch_cols * SLOT // 128   # 16 transposed blocks per chunk
    QCOLS = 512
    QSPL = 8         # q-build: DVE takes coef cols 0:QSPL, Pool the rest

    nc = bacc.Bacc(None, target_bir_lowering=False)

    # ---- DRAM I/O -------------------------------------------------------
    c4_d = nc.dram_tensor("c4pm", [P, T, 4], dt.float32, kind="ExternalInput")
    ipm_d = nc.dram_tensor("ipm", [P, T], dt.float32, kind="ExternalInput")
    patch_dt = dt.float16 if PATCH_F16 else dt.float32
    # per-core stack of per-chunk windows: [n_chunks, WIN, PTROW]
    ptw_d = nc.dram_tensor("ptw", [n_chunks * WIN, PTROW], patch_dt,
                           kind="ExternalInput")
    idx_d = nc.dram_tensor("idx16", [P, n_chunks * CHPTS // 16], dt.int16,
                           kind="ExternalInput")
    l1_d = nc.dram_tensor("lhsT1", [128, 128], dt.float16, kind="ExternalInput")
    l2_d = nc.dram_tensor("lhsT2", [128, 128], dt.float16, kind="ExternalInput")
    l3_d = nc.dram_tensor("lhsT3", [128, 128], dt.float16, kind="ExternalInput")
    b2_d = nc.dram_tensor("b2rep", [128, 1], dt.float32, kind="ExternalInput")
    b3_d = nc.dram_tensor("b3rep", [64, 1], dt.float32, kind="ExternalInput")
    out_d = nc.dram_tensor("yT", [64, B // 16], dt.float16, kind="ExternalOutput")

    with tile.TileContext(nc) as tc:
        with (
            tc.tile_pool(name="persist", bufs=1) as pp,
            tc.tile_pool(name="psum_l1", bufs=3, space="PSUM") as ps1,
            tc.tile_pool(name="psum_l2", bufs=3, space="PSUM") as ps2,
            tc.tile_pool(name="psum_l3", bufs=2, space="PSUM") as ps3,
            tc.tile_pool(name="mlp", bufs=8) as mp,
            tc.tile_pool(name="outp", bufs=2) as op_,
        ):
            f32 = dt.float32
            f16 = dt.float16

            idf = pp.tile([P, T], f32, tag="idf")
            c4 = pp.tile([P, T, 4], f32, tag="c4")
            # all gather indices staged in SBUF upfront: no ring coupling
            idxall = pp.tile([P, n_chunks * CHPTS // 16], dt.int16,
                             tag="idxall")
            l1w = pp.tile([128, 128], f16, tag="l1w")
            l2w = pp.tile([128, 128], f16, tag="l2w")
            l3w = pp.tile([128, 4, 32], f16, tag="l3w")
            b2r = pp.tile([128, 1], f32, tag="b2r")
            b3r = pp.tile([64, 1], f32, tag="b3r")

            NBUF = 6         # patch buffers, one per chunk
            patch = [pp.tile([P, ch_cols, GROW], patch_dt, name=f"patch{i}",
                             tag=f"patch{i}") for i in range(NBUF)]
            NQB = 4
            qpm = [pp.tile([P, ch_cols, SLOT], f16, name=f"qpm{i}",
                           tag=f"qpm{i}") for i in range(NQB)]
            qT = [pp.tile([P, n_blk, 128], f16, name=f"qT{i}", tag=f"qT{i}")
                  for i in range(NQB)]

            AL = mybir.AluOpType
            V = nc.vector
            nc.gpsimd.load_library(library_config.mlp)

            CW = CHPTS // 16     # idx columns per chunk
            nib = n_chunks * CW // 4
            cb = T // 4

            def load_blk(blk):
                """Stage 1/4 of the idx / c4 / idf inputs (4 chunks each)."""
                nc.sync.dma_start(idxall[:, blk * nib:(blk + 1) * nib],
                                  idx_d[:, blk * nib:(blk + 1) * nib])
                nc.sync.dma_start(c4[:, blk * cb:(blk + 1) * cb, :],
                                  c4_d[:, blk * cb:(blk + 1) * cb, :])
                nc.sync.dma_start(idf[:, blk * cb:(blk + 1) * cb],
                                  ipm_d[:, blk * cb:(blk + 1) * cb])

            for i in range(NQB):
                nc.vector.memset(qpm[i][:, :, 17:18], 1.0)
                nc.vector.memset(qpm[i][:, :, 18:SLOT], 0.0)

            def issue_gather(ci):
                # two 8192-idx gathers per chunk (ucode-validated shape),
                # sharing the chunk's 16384-row window
                for h in range(2):
                    _dma_gather_strided(
                        nc.gpsimd,
                        out_ap=patch[ci % NBUF][:, h * 64:(h + 1) * 64, :],
                        in_ap=ptw_d[ci * WIN:(ci + 1) * WIN, 0:GROW],
                        idxs_ap=idxall[:, ci * CW + h * (CW // 2):
                                       ci * CW + (h + 1) * (CW // 2)],
                        num_idxs=CHPTS // 2,
                        elem_size=GROW,
                        elem_step=PTROW,
                    )

            def build_q(ci):
                """q = coef * patch (-> fp16), split DVE / Pool; gather
                layout: point rank = g*128 + p -> dest[p, g, :]; idf/c4 are
                laid out in the same column-major rank order on host. Ends
                with the xbar transpose into qT."""
                pb = patch[ci % NBUF]
                qb = qpm[ci % NQB]
                c0 = ci * ch_cols
                cbc = c4[:, c0:c0 + ch_cols, :].unsqueeze(3).to_broadcast(
                    [P, ch_cols, 4, 4])
                qb4 = qb[:, :, 0:16].rearrange("p t (c f) -> p t c f", c=4)
                pb4 = pb[:, :, 0:16].rearrange("p t (c f) -> p t c f", c=4)
                cs = (QSPL // 4) if POOL_OPS else 4
                nc.vector.tensor_tensor(
                    qb4[:, :, 0:cs, :], pb4[:, :, 0:cs, :], cbc[:, :, 0:cs, :],
                    AL.mult)
                if POOL_OPS:
                    nc.gpsimd.tensor_tensor(
                        qb4[:, :, cs:4, :], pb4[:, :, cs:4, :],
                        cbc[:, :, cs:4, :], AL.mult)
                    nc.gpsimd.tensor_copy(qb[:, :, 16:17],
                                          idf[:, c0:c0 + ch_cols].unsqueeze(2))
                else:
                    nc.vector.tensor_copy(qb[:, :, 16:17],
                                          idf[:, c0:c0 + ch_cols].unsqueeze(2))
                nc.sync.dma_start_transpose(
                    qT[ci % NQB][:], qb[:].rearrange("p t s -> p (t s)"))

            PFD = 3
            # staged prologue: chunk 0's inputs and q-build first, so the
            # pipeline head isn't gated on the full input upload
            load_blk(0)
            issue_gather(0)
            build_q(0)
            nc.sync.dma_start(l1w[:], l1_d[:])
            nc.sync.dma_start(l2w[:], l2_d[:])
            nc.sync.dma_start(l3w[:], l3_d[:])
            nc.sync.dma_start(b2r[:], b2_d[:])
            nc.sync.dma_start(b3r[:], b3_d[:])
            for pre in range(1, min(PFD, n_chunks)):
                issue_gather(pre)
            for blk in range(1, 4):
                load_blk(blk)

            PERIOD_MS = float(os.environ.get("KPERIOD", "0.014"))
            for chi in range(n_chunks):
                tb = qT[chi % NQB]

                # scheduler-time floor: stops the list scheduler (whose DMA
                # model underestimates gather latency) from hoisting future
                # q-builds ahead of this chunk's relu stream
                with tc.tile_wait_until(PERIOD_MS * chi, enable=chi > 0):
                    if chi + PFD < n_chunks:
                        issue_gather(chi + PFD)
                    if chi + 1 < n_chunks:
                        build_q(chi + 1)

                y3c = op_.tile([64, 2 * QCOLS], f16, tag="y3c")
                # 16 units of 512 cols: u = 8*uh + uu, uu = 2*qq' + hh;
                # relu engines alternate whole-unit between DVE and ACT;
                # one sigmoid per 8-unit half from packed PSUM [64, 512]
                for uh in range(2):
                    l3p = ps3.tile([64, QCOLS], f32, tag="l3p")
                    for uu in range(8):
                        u = 8 * uh + uu
                        qq = 4 * uh + uu // 2
                        hh = uu % 2
                        rhs = tb[64 * hh:64 * hh + 64, 4 * qq:4 * qq + 4, :]
                        l1p = ps1.tile([P, QCOLS], f32, tag="l1p")
                        nc.tensor.matmul(l1p[:], l1w[64 * hh:64 * hh + 64], rhs,
                                         start=True, stop=True,
                                         tile_position=(64 * hh, 0))
                        h1 = mp.tile([P, QCOLS], f16, tag="h1")
                        if u % 2 == 0:
                            nc.vector.tensor_scalar(h1[:], l1p[:],
                                                    0.0, None, AL.max)
                        else:
                            nc.scalar.activation(
                                h1[:], l1p[:],
                                mybir.ActivationFunctionType.Relu)

                        l2p = ps2.tile([P, QCOLS], f32, tag="l2p")
                        nc.tensor.matmul(l2p[:], l2w[:], h1[:],
                                         start=True, stop=True)
                        h2 = mp.tile([P, QCOLS], f16, tag="h2")
                        if u % 2 == 1:
                            nc.vector.tensor_scalar(h2[:], l2p[:],
                                                    b2r[:], 0.0, AL.add, AL.max)
                        else:
                            nc.scalar.activation(
                                h2[:], l2p[:],
                                mybir.ActivationFunctionType.Relu, bias=b2r[:])

                        G, m = uu // 4, uu % 4
                        nc.tensor.matmul(l3p[32 * G:32 * G + 32, 0:QCOLS],
                                         l3w[:, m, :], h2[:],
                                         start=(m == 0), stop=(m == 3))
                    nc.scalar.activation(y3c[:, uh * QCOLS:(uh + 1) * QCOLS],
                                         l3p[:],
                                         mybir.ActivationFunctionType.Sigmoid,
                                         bias=b3r[:])
                nc.scalar.dma_start(
                    out_d[:, chi * 2 * QCOLS:(chi + 1) * 2 * QCOLS], y3c[:])

    return nc


def _host_cells(x):
    """Exact fp32 replica of the device's cell computation (int routing)."""
    u = x[:, 1].astype(np.float32)
    v = x[:, 2].astype(np.float32)
    xu = (u * np.float32(RX)).astype(np.float32)
    yv = (v * np.float32(RY)).astype(np.float32)
    x0 = np.floor(xu)
    y0 = np.floor(yv)
    x0 = np.where(x0 == RX, 0.0, x0).astype(np.float32)
    cell = np.minimum(y0 * RX + x0, RX * RY - 1).astype(np.int64)
    return cell


def _host_c4(x):
    """Bilinear coefficients [N, 4] in fp32, replicating the reference's
    weight arithmetic (wx computed after the x0==RX wrap, trunc as floor
    since u,v >= 0)."""
    f32 = np.float32
    u = x[:, 1].astype(f32)
    v = x[:, 2].astype(f32)
    xu = u * f32(RX)
    yv = v * f32(RY)
    x0 = np.trunc(xu).astype(f32)
    y0 = np.trunc(yv).astype(f32)
    x0w = np.where(x0 == f32(RX), f32(0.0), x0)
    wx = (xu - x0w).astype(f32)
    wy = (yv - y0).astype(f32)
    bx = (f32(1.0) - wx).astype(f32)
    by = (f32(1.0) - wy).astype(f32)
    c00 = (bx * by).astype(f32)
    c10 = (by - c00).astype(f32)
    c01 = (bx - c00).astype(f32)
    c11 = (wx - c10).astype(f32)
    return np.stack([c00, c10, c01, c11], axis=1)


def _host_prep_weights(w1, b1, w2, b2, w3, b3):
    w1 = np.asarray(w1, np.float32)
    b1 = np.asarray(b1, np.float32)
    w1x = np.zeros((QROWS, HID), np.float32)
    for c in range(4):
        w1x[4 * c:4 * c + 4, :] = w1[1:5, :]
    w1x[16, :] = w1[0, :]
    w1x[17, :] = b1
    lhsT1 = np.zeros((128, 128), np.float16)
    lhsT1[0:QROWS, 0:64] = w1x
    lhsT1[32:32 + QROWS, 64:128] = w1x
    lhsT1[64:128, :] = lhsT1[0:64, :]
    lhsT2 = np.zeros((128, 128), np.float16)
    lhsT2[0:64, 0:64] = w2
    lhsT2[64:128, 64:128] = w2
    lhsT3 = np.zeros((128, 4, 32), np.float16)
    for m in range(4):
        lhsT3[0:64, m, 8 * m:8 * m + 3] = w3
        lhsT3[64:128, m, 8 * m + 3:8 * m + 6] = w3
    lhsT3 = lhsT3.reshape(128, 128)
    b2rep = np.concatenate([b2, b2]).astype(np.float32).reshape(128, 1)
    b3rep = np.zeros((64, 1), np.float32)
    for g in range(8):
        b3rep[8 * g:8 * g + 3, 0] = b3
        b3rep[8 * g + 3:8 * g + 6, 0] = b3
    return lhsT1, lhsT2, lhsT3, b2rep, b3rep


def _patch_table(emb):
    e = np.asarray(emb, dtype=np.float32).reshape(RY, RX, F)
    xs = np.arange(RX)
    x1 = np.minimum(xs + 1, RX - 1)
    ys = np.arange(RY)
    y1 = np.minimum(ys + 1, RY - 1)
    pt = np.zeros((RY, RX, PTROW),
                  dtype=np.float16 if PATCH_F16 else np.float32)
    pt[:, :, 0:F] = e
    pt[:, :, F:2 * F] = e[:, x1, :]
    pt[:, :, 2 * F:3 * F] = e[y1, :, :]
    pt[:, :, 3 * F:4 * F] = e[y1][:, x1, :]
    return np.ascontiguousarray(pt.reshape(RX * RY, PTROW))


def _out_scatter(B):
    """Flat scatter indices for the packed yT [64, B/16] fp16 output.

    Row r = 8*g2 + j (j<6 used: j//3 = pair half, j%3 = output dim).
    Column chi*1024 + uh*512 + n: unit half uh, quarter qq = 4*uh + g2//2,
    hh = g2%2, block b = 4*qq + n//128, lane p = n%128; point col
    t = 4*b + 2*hh + j//3; local rank = t*128 + p within chunk chi.
    Returns idx [64, n_chunks*1024] into the flat [B*3] per-core output
    (entries for j>=6 point at a scratch slot B*3)."""
    n_chunks = B // CHPTS
    g2 = np.arange(8)[:, None, None, None, None]
    j = np.arange(8)[None, :, None, None, None]
    chi = np.arange(n_chunks)[None, None, :, None, None]
    uh = np.arange(2)[None, None, None, :, None]
    n = np.arange(QC)[None, None, None, None, :]
    qq = 4 * uh + g2 // 2
    hh = g2 % 2
    t = 4 * (4 * qq + n // 128) + 2 * hh + np.minimum(j, 5) // 3
    rank = chi * CHPTS + t * P + (n % 128)
    idx = rank * 3 + np.minimum(j, 5) % 3
    idx = np.where(j >= 6, B * 3, idx)
    return idx.reshape(64, n_chunks * 2 * QC)



def _prep_in_maps(x, emb, w1, b1, w2, b2, w3, b3, n_cores):
    x = np.asarray(x, np.float32)
    N = x.shape[0]
    B = N // n_cores
    T = B // P
    n_chunks = B // CHPTS
    cell = _host_cells(x)
    order = np.argsort(cell, kind="stable")
    cell_s = cell[order]
    xs = x[order]
    c4s = _host_c4(x)[order]
    pt = _patch_table(emb)
    lhsT1, lhsT2, lhsT3, b2rep, b3rep = _host_prep_weights(w1, b1, w2, b2, w3, b3)
    in_maps = []
    for k in range(n_cores):
        ci = cell_s[k * B:(k + 1) * B]
        xc = xs[k * B:(k + 1) * B]
        ptw = np.empty((n_chunks * WIN, PTROW),
                       np.float16 if PATCH_F16 else np.float32)
        idx16 = np.empty((P, n_chunks * CHPTS // 16), np.int16)
        for c in range(n_chunks):
            cc = ci[c * CHPTS:(c + 1) * CHPTS]
            base = int(np.clip((int(cc[0]) + int(cc[-1]) + 1) // 2 - WIN // 2,
                               0, RX * RY - WIN))
            lo = cc - base
            assert lo.min() >= 0 and lo.max() < WIN, (
                f"window miss core {k} chunk {c}: {lo.min()} {lo.max()}")
            ptw[c * WIN:(c + 1) * WIN] = pt[base:base + WIN]
            w16 = lo.astype(np.int16).reshape(CHPTS // 16, 16).T
            idx16[:, c * (CHPTS // 16):(c + 1) * (CHPTS // 16)] = np.tile(w16, (8, 1))
        c4c = c4s[k * B:(k + 1) * B]
        in_maps.append({
            "c4pm": np.ascontiguousarray(
                c4c.reshape(T, P, 4).transpose(1, 0, 2)),
            "ipm": np.ascontiguousarray(xc[:, 0].reshape(T, P).T),
            "ptw": ptw,
            "idx16": idx16,
            "lhsT1": lhsT1,
            "lhsT2": lhsT2,
            "lhsT3": lhsT3,
            "b2rep": b2rep,
            "b3rep": b3rep,
        })
    return in_maps, order


_CACHE = {}


def kernel(x, emb, w1, b1, w2, b2, w3, b3):
    from concourse.bass_utils import run_bass_kernel_spmd

    x = np.asarray(x, np.float32)
    N = x.shape[0]
    B = N // N_CORES
    T = B // P
    n_chunks = B // CHPTS

    in_maps, order = _prep_in_maps(x, emb, w1, b1, w2, b2, w3, b3, n_cores=N_CORES)

    key = (B,)
    if key not in _CACHE:
        nc_new = _build_bass(B, N_CORES, None)
        nc_new.compile()
        _CACHE[key] = nc_new
    nc = _CACHE[key]

    trace = os.environ.get("KERNEL_TRACE", "0") == "1"
    res = run_bass_kernel_spmd(
        nc, in_maps, core_ids=list(range(N_CORES)), trace=trace
    )
    if trace and res.exec_time_ns is not None:
        print(f"HW exec time: {res.exec_time_ns} ns")

    sc = _out_scatter(B).ravel()
    y_sorted = np.empty((N, 3), np.float32)
    for k in range(N_CORES):
        yT = res.results[k]["yT"]
        buf = np.empty(B * 3 + 1, np.float32)
        buf[sc] = yT.reshape(64, -1).ravel().astype(np.float32)
        y_sorted[k * B:(k + 1) * B, :] = buf[:B * 3].reshape(B, 3)
    y = np.empty((N, 3), np.float32)
    y[order, :] = y_sorted
    return y



# revision 94
# speedup vs baseline: 1.1591x; 1.1591x over previous
# kernel.py — Trainium2 Bass kernel for nn_DenseGridNet (bilinear grid sample + MLP)
#
# Strategy (data-parallel over 8 NeuronCores, sorted sharding):
#  * host: computes cell ids + bilinear coefficients c4 in exact fp32, sorts
#    points by cell (routing only), builds a 256B-strided fp16 patch table
#    PT[cell] = [v00,v10,v01,v11] and int16 window-local gather indices
#    (16384-row windows per 16K-point chunk, fixed at compile time).
#  * device, per 16K-point chunk: two 8192-idx dma_gathers fetch 32B patch
#    payloads from the 256B-strided window (payload < stride via a direct
#    InstDMAGatherAnt build — descriptor-floor cost instead of the 2x small-
#    transfer penalty); DVE+Pool build fp16 q = c4*patch (+idf+ones slots);
#    one xbar DMA transpose flips point-major -> feature-major; TensorE runs
#    the 3 MLP layers as 16 block-diagonal 512-col units (2 pts/col, 1 PSUM
#    bank each, 3-deep rotation); relu PSUM->SBUF passes alternate whole-unit
#    between DVE and ACT; layer-3 outputs of 4 units accumulate into one
#    [64,512] PSUM tile via offset lhsT blocks so ONE sigmoid covers them;
#    fp16 output yT [64, B/16] is un-permuted on host.
#  * scheduling: Tile's list scheduler is steered with tile_wait_until floors
#    (per-chunk cadence ~13.5us) so its optimistic DMA model can't hoist
#    future q-builds ahead of the current chunk's relu stream.
import os
import numpy as np

RX = 1024
RY = 1024
F = 4
HID = 64
N_CORES = 8
P = 128          # partitions
SLOT = 32        # fp16 columns per point in the pre-transpose buffer
QROWS = 18       # used rows per point: 16 q + idf + ones
WIN = 16384      # dma_gather window rows per chunk (int16-addressable)
CHPTS = 16384    # points per gather/transpose/MLP chunk
PATCH_F16 = True     # patch table dtype (False -> fp32, 64B payload)
POOL_OPS = True      # offload part of q-build + idf copy to the Pool engine
PTROW = 128 if PATCH_F16 else 64   # patch row STRIDE in elements (256B)
GROW = 16        # gathered payload per point, elements
QC = 512         # columns per sigmoid group in the packed yT layout


def _dma_gather_strided(g, out_ap, in_ap, idxs_ap, num_idxs, elem_size, elem_step):
    """dma_gather with elem_size smaller than the 256B-multiple the bass
    wrapper asserts. The hardware encodes the source stride separately
    (stride_bytes_256, 256B units) from the transfer size (elem_size), so a
    64B payload gathered from 256B-strided rows is representable; only the
    wrapper's transpose-mode assert blocks it. Mirrors BassGpSimd.dma_gather
    (non-transpose, DRAM source, gen_mode=0)."""
    import concourse.mybir as mybir
    from concourse import ap_utils

    assert idxs_ap.dtype == mybir.dt.int16
    assert in_ap.dtype == out_ap.dtype
    assert in_ap.ap[0][0] == elem_step
    assert in_ap.ap[-1][1] == out_ap.ap[-1][1] == elem_size
    assert out_ap.ap[0][1] * out_ap.ap[1][1] * out_ap.ap[2][1] // elem_size >= num_idxs
    stride_bytes = elem_step * mybir.dt.size(in_ap.dtype)
    assert stride_bytes % 256 == 0
    stride_bytes_256 = stride_bytes // 256
    _in_ap = g.lower_ap_dma(in_ap, for_custom_bir_dma=True)
    _idxs_ap = g.lower_ap(idxs_ap)
    _out_ap = g.lower_ap(out_ap)
    return g.add_instruction(
        mybir.InstDMAGatherAnt(
            name=g.bass.get_next_instruction_name(),
            ins=[*_in_ap, _idxs_ap, g.lower_val_access(g.to_reg(num_idxs))],
            outs=[_out_ap],
            transpose=False,
            num_idxs=num_idxs,
            elem_size=elem_size,
            stride_bytes_256=stride_bytes_256,
            gen_mode=0,
            single_packet=False,
            queue_num=0,
            sbuf_tokens_per_rank=0,
            sbuf_free_dim_per_rank=0,
            sbuf_free_dim_pad_per_rank=0,
            sbuf_byte_offset=0,
        )
    )


def _build_bass(B, n_cores, core0_chunk0):
    """Bass program for one core processing B sorted points.

    core0_chunk0: list of per-core first-global-chunk indices is not needed;
    windows differ per core, but SPMD shares one program — so window bases
    must be identical across cores. Instead the table input is pre-shifted
    per core on host: in_ ap uses window base relative to a per-core table
    slice. We pass windows as compile-time offsets into the per-core table
    input "ptw" which the host fills with the right 32768-row windows.
    """
    import concourse.bass as bass
    import concourse.tile as tile
    from concourse import bacc, library_config
    import concourse.mybir as mybir

    dt = mybir.dt
    T = B // P
    n_chunks = B // CHPTS
    ch_cols = CHPTS // P            # 64 point cols per chunk
    n_blk = ch_cols * SLOT // 128   # 16 transposed blocks per chunk
    QCOLS = 512
    QSPL = int(os.environ.get("KQSPL", "8"))

    nc = bacc.Bacc(None, target_bir_lowering=False)

    # ---- DRAM I/O -------------------------------------------------------
    c4_d = nc.dram_tensor("c4pm", [P, T, 4], dt.float16, kind="ExternalInput")
    ipm_d = nc.dram_tensor("ipm", [P, T], dt.float16, kind="ExternalInput")
    patch_dt = dt.float16 if PATCH_F16 else dt.float32
    # per-core stack of per-chunk windows: [n_chunks, WIN, PTROW]
    ptw_d = nc.dram_tensor("ptw", [n_chunks * WIN, PTROW], patch_dt,
                           kind="ExternalInput")
    idx_d = nc.dram_tensor("idx16", [P, n_chunks * CHPTS // 16], dt.int16,
                           kind="ExternalInput")
    l1_d = nc.dram_tensor("lhsT1", [128, 128], dt.float16, kind="ExternalInput")
    l2_d = nc.dram_tensor("lhsT2", [128, 128], dt.float16, kind="ExternalInput")
    l3_d = nc.dram_tensor("lhsT3", [128, 128], dt.float16, kind="ExternalInput")
    b2_d = nc.dram_tensor("b2rep", [128, 1], dt.float32, kind="ExternalInput")
    b3_d = nc.dram_tensor("b3rep", [64, 1], dt.float32, kind="ExternalInput")
    out_d = nc.dram_tensor("yT", [64, B // 16], dt.float16, kind="ExternalOutput")

    with tile.TileContext(nc) as tc:
        with (
            tc.tile_pool(name="persist", bufs=1) as pp,
            tc.tile_pool(name="psum_l1",
                         bufs=int(os.environ.get("KPS1", "3")),
                         space="PSUM") as ps1,
            tc.tile_pool(name="psum_l2",
                         bufs=int(os.environ.get("KPS2", "3")),
                         space="PSUM") as ps2,
            tc.tile_pool(name="psum_l3", bufs=2, space="PSUM") as ps3,
            tc.tile_pool(name="mlp",
                         bufs=int(os.environ.get("KMPB", "8"))) as mp,
            tc.tile_pool(name="outp",
                         bufs=int(os.environ.get("KOPB", "2"))) as op_,
        ):
            f32 = dt.float32
            f16 = dt.float16

            idf = pp.tile([P, T], f16, tag="idf")
            c4 = pp.tile([P, T, 4], f16, tag="c4")
            # all gather indices staged in SBUF upfront: no ring coupling
            idxall = pp.tile([P, n_chunks * CHPTS // 16], dt.int16,
                             tag="idxall")
            l1w = pp.tile([128, 128], f16, tag="l1w")
            l2w = pp.tile([128, 128], f16, tag="l2w")
            l3w = pp.tile([128, 4, 32], f16, tag="l3w")
            b2r = pp.tile([128, 1], f32, tag="b2r")
            b3r = pp.tile([64, 1], f32, tag="b3r")

            NBUF = int(os.environ.get("KNBUF", "6"))
            patch = [pp.tile([P, ch_cols, GROW], patch_dt, name=f"patch{i}",
                             tag=f"patch{i}") for i in range(NBUF)]
            NQB = int(os.environ.get("KNQB", "5"))
            qpm = [pp.tile([P, ch_cols, SLOT], f16, name=f"qpm{i}",
                           tag=f"qpm{i}") for i in range(NQB)]
            qT = [pp.tile([P, n_blk, 128], f16, name=f"qT{i}", tag=f"qT{i}")
                  for i in range(NQB)]

            AL = mybir.AluOpType
            V = nc.vector
            nc.gpsimd.load_library(library_config.mlp)

            CW = CHPTS // 16     # idx columns per chunk
            nib = n_chunks * CW // 4
            cb = T // 4

            def load_blk(blk):
                """Stage 1/4 of the idx / c4 / idf inputs (4 chunks each)."""
                lo = blk * nib + (CW if blk == 0 else 0)
                nc.sync.dma_start(idxall[:, lo:(blk + 1) * nib],
                                  idx_d[:, lo:(blk + 1) * nib])
                nc.sync.dma_start(c4[:, blk * cb:(blk + 1) * cb, :],
                                  c4_d[:, blk * cb:(blk + 1) * cb, :])
                nc.sync.dma_start(idf[:, blk * cb:(blk + 1) * cb],
                                  ipm_d[:, blk * cb:(blk + 1) * cb])

            for i in range(NQB):
                nc.vector.memset(qpm[i][:, :, 17:18], 1.0)
                nc.vector.memset(qpm[i][:, :, 18:SLOT], 0.0)

            def issue_gather(ci):
                # two 8192-idx gathers per chunk (ucode-validated shape),
                # sharing the chunk's 16384-row window
                for h in range(2):
                    _dma_gather_strided(
                        nc.gpsimd,
                        out_ap=patch[ci % NBUF][:, h * 64:(h + 1) * 64, :],
                        in_ap=ptw_d[ci * WIN:(ci + 1) * WIN, 0:GROW],
                        idxs_ap=idxall[:, ci * CW + h * (CW // 2):
                                       ci * CW + (h + 1) * (CW // 2)],
                        num_idxs=CHPTS // 2,
                        elem_size=GROW,
                        elem_step=PTROW,
                    )

            def build_q(ci):
                """q = coef * patch (-> fp16), split DVE / Pool; gather
                layout: point rank = g*128 + p -> dest[p, g, :]; idf/c4 are
                laid out in the same column-major rank order on host. Ends
                with the xbar transpose into qT."""
                pb = patch[ci % NBUF]
                qb = qpm[ci % NQB]
                c0 = ci * ch_cols
                cbc = c4[:, c0:c0 + ch_cols, :].unsqueeze(3).to_broadcast(
                    [P, ch_cols, 4, 4])
                qb4 = qb[:, :, 0:16].rearrange("p t (c f) -> p t c f", c=4)
                pb4 = pb[:, :, 0:16].rearrange("p t (c f) -> p t c f", c=4)
                cs = (QSPL // 4) if POOL_OPS else 4
                nc.vector.tensor_tensor(
                    qb4[:, :, 0:cs, :], pb4[:, :, 0:cs, :], cbc[:, :, 0:cs, :],
                    AL.mult)
                if POOL_OPS:
                    nc.gpsimd.tensor_tensor(
                        qb4[:, :, cs:4, :], pb4[:, :, cs:4, :],
                        cbc[:, :, cs:4, :], AL.mult)
                    nc.gpsimd.tensor_copy(qb[:, :, 16:17],
                                          idf[:, c0:c0 + ch_cols].unsqueeze(2))
                else:
                    nc.vector.tensor_copy(qb[:, :, 16:17],
                                          idf[:, c0:c0 + ch_cols].unsqueeze(2))
                nc.sync.dma_start_transpose(
                    qT[ci % NQB][:], qb[:].rearrange("p t s -> p (t s)"))

            PFD = int(os.environ.get("KPFD", "2"))
            # staged prologue: chunk 0's idx columns load first (0.36us) so
            # gather 0's desc-gen starts immediately; then the rest
            nc.sync.dma_start(idxall[:, 0:CW], idx_d[:, 0:CW])
            load_blk0_rest = True
            load_blk(0)
            issue_gather(0)
            build_q(0)
            nc.sync.dma_start(l1w[:], l1_d[:])
            nc.sync.dma_start(l2w[:], l2_d[:])
            nc.sync.dma_start(l3w[:], l3_d[:])
            nc.sync.dma_start(b2r[:], b2_d[:])
            nc.sync.dma_start(b3r[:], b3_d[:])
            for pre in range(1, min(PFD, n_chunks)):
                with tc.tile_wait_until(0.004 * pre):
                    issue_gather(pre)
            for blk in range(1, 4):
                load_blk(blk)

            PERIOD_MS = float(os.environ.get("KPERIOD", "0.0135"))
            OFF_MS = float(os.environ.get("KOFF", "0.0095"))
            for chi in range(n_chunks):
                tb = qT[chi % NQB]

                # scheduler-time floor: stops the list scheduler (whose DMA
                # model underestimates gather latency) from hoisting future
                # q-builds ahead of this chunk's relu stream
                QLAG = float(os.environ.get("KQLAG", "0.0"))
                with tc.tile_wait_until(OFF_MS + PERIOD_MS * chi,
                                        enable=chi > 0):
                    if chi + PFD < n_chunks:
                        issue_gather(chi + PFD)
                with tc.tile_wait_until(OFF_MS + PERIOD_MS * (chi + QLAG),
                                        enable=chi > 0):
                    if chi + 1 < n_chunks:
                        build_q(chi + 1)

                y3c = op_.tile([64, 2 * QCOLS], f16, tag="y3c")
                # 16 units of 512 cols: u = 8*uh + uu, uu = 2*qq' + hh;
                # relu engines alternate whole-unit between DVE and ACT;
                # one sigmoid per 8-unit half from packed PSUM [64, 512]
                for uh in range(2):
                    l3p = ps3.tile([64, QCOLS], f32, tag="l3p")
                    for uu in range(8):
                        u = 8 * uh + uu
                        qq = 4 * uh + uu // 2
                        hh = uu % 2
                        rhs = tb[64 * hh:64 * hh + 64, 4 * qq:4 * qq + 4, :]
                        l1p = ps1.tile([P, QCOLS], f32, tag="l1p")
                        nc.tensor.matmul(l1p[:], l1w[64 * hh:64 * hh + 64], rhs,
                                         start=True, stop=True,
                                         tile_position=(64 * hh, 0))
                        h1 = mp.tile([P, QCOLS], f16, tag="h1")
                        if u % 2 == 0:
                            nc.vector.tensor_scalar(h1[:], l1p[:],
                                                    0.0, None, AL.max)
                        else:
                            nc.scalar.activation(
                                h1[:], l1p[:],
                                mybir.ActivationFunctionType.Relu)

                        l2p = ps2.tile([P, QCOLS], f32, tag="l2p")
                        nc.tensor.matmul(l2p[:], l2w[:], h1[:],
                                         start=True, stop=True)
                        h2 = mp.tile([P, QCOLS], f16, tag="h2")
                        if u % 2 == 1:
                            nc.vector.tensor_scalar(h2[:], l2p[:],
                                                    b2r[:], 0.0, AL.add, AL.max)
                        else:
                            nc.scalar.activation(
                                h2[:], l2p[:],
                                mybir.ActivationFunctionType.Relu, bias=b2r[:])

                        G, m = uu // 4, uu % 4
                        nc.tensor.matmul(l3p[32 * G:32 * G + 32, 0:QCOLS],
                                         l3w[:, m, :], h2[:],
                                         start=(m == 0), stop=(m == 3))
                    nc.scalar.activation(y3c[:, uh * QCOLS:(uh + 1) * QCOLS],
                                         l3p[:],
                                         mybir.ActivationFunctionType.Sigmoid,
                                         bias=b3r[:])
                nc.scalar.dma_start(
                    out_d[:, chi * 2 * QCOLS:(chi + 1) * 2 * QCOLS], y3c[:])

    return nc


def _host_cells(x):
    """Exact fp32 replica of the device's cell computation (int routing)."""
    u = x[:, 1].astype(np.float32)
    v = x[:, 2].astype(np.float32)
    xu = (u * np.float32(RX)).astype(np.float32)
    yv = (v * np.float32(RY)).astype(np.float32)
    x0 = np.floor(xu)
    y0 = np.floor(yv)
    x0 = np.where(x0 == RX, 0.0, x0).astype(np.float32)
    cell = np.minimum(y0 * RX + x0, RX * RY - 1).astype(np.int64)
    return cell


def _host_c4(x):
    """Bilinear coefficients [N, 4] in fp32, replicating the reference's
    weight arithmetic (wx computed after the x0==RX wrap, trunc as floor
    since u,v >= 0)."""
    f32 = np.float32
    u = x[:, 1].astype(f32)
    v = x[:, 2].astype(f32)
    xu = u * f32(RX)
    yv = v * f32(RY)
    x0 = np.trunc(xu).astype(f32)
    y0 = np.trunc(yv).astype(f32)
    x0w = np.where(x0 == f32(RX), f32(0.0), x0)
    wx = (xu - x0w).astype(f32)
    wy = (yv - y0).astype(f32)
    bx = (f32(1.0) - wx).astype(f32)
    by = (f32(1.0) - wy).astype(f32)
    c00 = (bx * by).astype(f32)
    c10 = (by - c00).astype(f32)
    c01 = (bx - c00).astype(f32)
    c11 = (wx - c10).astype(f32)
    return np.stack([c00, c10, c01, c11], axis=1)


def _host_prep_weights(w1, b1, w2, b2, w3, b3):
    w1 = np.asarray(w1, np.float32)
    b1 = np.asarray(b1, np.float32)
    w1x = np.zeros((QROWS, HID), np.float32)
    for c in range(4):
        w1x[4 * c:4 * c + 4, :] = w1[1:5, :]
    w1x[16, :] = w1[0, :]
    w1x[17, :] = b1
    lhsT1 = np.zeros((128, 128), np.float16)
    lhsT1[0:QROWS, 0:64] = w1x
    lhsT1[32:32 + QROWS, 64:128] = w1x
    lhsT1[64:128, :] = lhsT1[0:64, :]
    lhsT2 = np.zeros((128, 128), np.float16)
    lhsT2[0:64, 0:64] = w2
    lhsT2[64:128, 64:128] = w2
    lhsT3 = np.zeros((128, 4, 32), np.float16)
    for m in range(4):
        lhsT3[0:64, m, 8 * m:8 * m + 3] = w3
        lhsT3[64:128, m, 8 * m + 3:8 * m + 6] = w3
    lhsT3 = lhsT3.reshape(128, 128)
    b2rep = np.concatenate([b2, b2]).astype(np.float32).reshape(128, 1)
    b3rep = np.zeros((64, 1), np.float32)
    for g in range(8):
        b3rep[8 * g:8 * g + 3, 0] = b3
        b3rep[8 * g + 3:8 * g + 6, 0] = b3
    return lhsT1, lhsT2, lhsT3, b2rep, b3rep


def _patch_table(emb):
    e = np.asarray(emb, dtype=np.float32).reshape(RY, RX, F)
    xs = np.arange(RX)
    x1 = np.minimum(xs + 1, RX - 1)
    ys = np.arange(RY)
    y1 = np.minimum(ys + 1, RY - 1)
    pt = np.zeros((RY, RX, PTROW),
                  dtype=np.float16 if PATCH_F16 else np.float32)
    pt[:, :, 0:F] = e
    pt[:, :, F:2 * F] = e[:, x1, :]
    pt[:, :, 2 * F:3 * F] = e[y1, :, :]
    pt[:, :, 3 * F:4 * F] = e[y1][:, x1, :]
    return np.ascontiguousarray(pt.reshape(RX * RY, PTROW))


def _out_scatter(B):
    """Flat scatter indices for the packed yT [64, B/16] fp16 output.

    Row r = 8*g2 + j (j<6 used: j//3 = pair half, j%3 = output dim).
    Column chi*1024 + uh*512 + n: unit half uh, quarter qq = 4*uh + g2//2,
    hh = g2%2, block b = 4*qq + n//128, lane p = n%128; point col
    t = 4*b + 2*hh + j//3; local rank = t*128 + p within chunk chi.
    Returns idx [64, n_chunks*1024] into the flat [B*3] per-core output
    (entries for j>=6 point at a scratch slot B*3)."""
    n_chunks = B // CHPTS
    g2 = np.arange(8)[:, None, None, None, None]
    j = np.arange(8)[None, :, None, None, None]
    chi = np.arange(n_chunks)[None, None, :, None, None]
    uh = np.arange(2)[None, None, None, :, None]
    n = np.arange(QC)[None, None, None, None, :]
    qq = 4 * uh + g2 // 2
    hh = g2 % 2
    t = 4 * (4 * qq + n // 128) + 2 * hh + np.minimum(j, 5) // 3
    rank = chi * CHPTS + t * P + (n % 128)
    idx = rank * 3 + np.minimum(j, 5) % 3
    idx = np.where(j >= 6, B * 3, idx)
    return idx.reshape(64, n_chunks * 2 * QC)



def _prep_in_maps(x, emb, w1, b1, w2, b2, w3, b3, n_cores):
    x = np.asarray(x, np.float32)
    N = x.shape[0]
    B = N // n_cores
    T = B // P
    n_chunks = B // CHPTS
    cell = _host_cells(x)
    order = np.argsort(cell, kind="stable")
    cell_s = cell[order]
    xs = x[order]
    c4s = _host_c4(x)[order]
    pt = _patch_table(emb)
    lhsT1, lhsT2, lhsT3, b2rep, b3rep = _host_prep_weights(w1, b1, w2, b2, w3, b3)
    in_maps = []
    for k in range(n_cores):
        ci = cell_s[k * B:(k + 1) * B]
        xc = xs[k * B:(k + 1) * B]
        ptw = np.empty((n_chunks * WIN, PTROW),
                       np.float16 if PATCH_F16 else np.float32)
        idx16 = np.empty((P, n_chunks * CHPTS // 16), np.int16)
        for c in range(n_chunks):
            cc = ci[c * CHPTS:(c + 1) * CHPTS]
            base = int(np.clip((int(cc[0]) + int(cc[-1]) + 1) // 2 - WIN // 2,
                               0, RX * RY - WIN))
            lo = cc - base
            assert lo.min() >= 0 and lo.max() < WIN, (
                f"window miss core {k} chunk {c}: {lo.min()} {lo.max()}")
            ptw[c * WIN:(c + 1) * WIN] = pt[base:base + WIN]
            w16 = lo.astype(np.int16).reshape(CHPTS // 16, 16).T
            idx16[:, c * (CHPTS // 16):(c + 1) * (CHPTS // 16)] = np.tile(w16, (8, 1))
        c4c = c4s[k * B:(k + 1) * B]
        in_maps.append({
            "c4pm": np.ascontiguousarray(
                c4c.reshape(T, P, 4).transpose(1, 0, 2)).astype(np.float16),
            "ipm": np.ascontiguousarray(
                xc[:, 0].reshape(T, P).T).astype(np.float16),
            "ptw": ptw,
            "idx16": idx16,
            "lhsT1": lhsT1,
            "lhsT2": lhsT2,
            "lhsT3": lhsT3,
            "b2rep": b2rep,
            "b3rep": b3rep,
        })
    return in_maps, order


_CACHE = {}


def kernel(x, emb, w1, b1, w2, b2, w3, b3):
    from concourse.bass_utils import run_bass_kernel_spmd

    x = np.asarray(x, np.float32)
    N = x.shape[0]
    B = N // N_CORES
    T = B // P
    n_chunks = B // CHPTS

    in_maps, order = _prep_in_maps(x, emb, w1, b1, w2, b2, w3, b3, n_cores=N_CORES)

    key = (B,)
    if key not in _CACHE:
        nc_new = _build_bass(B, N_CORES, None)
        nc_new.compile()
        _CACHE[key] = nc_new
    nc = _CACHE[key]

    trace = os.environ.get("KERNEL_TRACE", "0") == "1"
    res = run_bass_kernel_spmd(
        nc, in_maps, core_ids=list(range(N_CORES)), trace=trace
    )
    if trace and res.exec_time_ns is not None:
        print(f"HW exec time: {res.exec_time_ns} ns")

    sc = _out_scatter(B).ravel()
    y_sorted = np.empty((N, 3), np.float32)
    for k in range(N_CORES):
        yT = res.results[k]["yT"]
        buf = np.empty(B * 3 + 1, np.float32)
        buf[sc] = yT.reshape(64, -1).ravel().astype(np.float32)
        y_sorted[k * B:(k + 1) * B, :] = buf[:B * 3].reshape(B, 3)
    y = np.empty((N, 3), np.float32)
    y[order, :] = y_sorted
    return y

